# revision 29
# baseline (speedup 1.0000x reference)
"""MoE downsample kernel for 8 TRN2 NeuronCores — top-2 sparse.

The reference computes all 4 experts densely, but only the top-2 gated
experts per sample contribute to the output. Gating depends only on the
input mean-pool, so it is computed on host BEFORE compiling the device
program; the program then contains matmuls only for the selected
(sample, expert) pairs (~half the dense FLOPs for typical gatings).

Sharding: every selected (sample, expert) conv is band-sharded across
all 8 cores — core c computes output rows [16c, 16c+16). All cores
therefore execute an IDENTICAL instruction stream (SPMD-safe); only the
staged input rows differ per core. Within a core, samples are processed
in rounds streamed through SBUF: samples are exactly balanced
(subset-sum over gating costs) onto the two PE row-halves (partitions
0-63 / 64-127), and each sample's two experts are split across the two
PE col-halves so four 64x64 tile_position matmuls run concurrently
(full 128x128 array). Each strided dilated
conv is decomposed into k*k "tap" matmuls accumulated in PSUM over
512-pixel chunks; BN + conv-bias + GELU fuse into the ScalarE PSUM
eviction. Top-2 weighting and concat run on host.

The PE weight-load path (one LDWEIGHTS per matmul, ~53ns per 64-col
load) is co-saturated with the moving-data path when every 512-px
matmul reloads its tap weights. Middle rounds therefore run TAP-OUTER:
each queue processes its two chunks of an (s, e) pair per tap
back-to-back, and a post-schedule pass deletes the second (redundant)
LDWEIGHTS so each tap's weights are loaded once per queue instead of
twice. Round 0 stays chunk-outer so the first matmul only needs a few
staged rows; the last round stays chunk-outer to stagger the final
GELU evictions into the matmul stream (shorter tail).
"""

import numpy as np
import ml_dtypes

KS = [3, 5, 7, 9]
DS = [1, 2, 3, 4]
HALO = [d * (k - 1) // 2 for k, d in zip(KS, DS)]  # [1, 4, 9, 16]
BN_EPS = 1e-5
B, CIN, H, W = 16, 64, 256, 256
CE = 64
PAD = 16           # left/top pad (max halo); right/bottom needs 15
HP = WP = PAD + 256 + 15   # 287
HO = WO = 128
NCORES = 8
BAND = 16          # output rows per core per (sample, expert)
CHUNK_ROWS = 4     # output rows per 512-px PSUM chunk
NCHUNK = BAND // CHUNK_ROWS   # 4 chunks per (sample, expert) band
RMAX = 31 + 2 * max(HALO)     # 63 input rows per staged piece (max)
R2MAX = (RMAX + 1) // 2       # 32 rows per parity plane
W2MAX = 144                   # cols per parity plane (ceil(287/2) rounded)
NTAPS = sum(k * k for k in KS)  # 164
# chunk-outer everywhere: tap-outer + LDWEIGHTS dedup measured neutral
# on the PE (the weight path is not the binding resource), and its
# eviction bunching cost more than the reuse saved
TAP_OUTER_MIDDLE = False

# tap slot base per expert in the packed weight tensor
_SLOT_BASE = np.cumsum([0] + [k * k for k in KS]).tolist()

_CACHE = {}


def _piece_width(halo):
    """Staged piece columns: window cols span [0, 2*halo + 254]."""
    return 2 * halo + 255


def _plane_width(halo):
    """Per-parity-plane columns: ceil(piece_width / 2)."""
    return halo + 128


def _tap_offsets(e, halo):
    """Yield (slot, row_off, col_off) in piece coords for expert e.

    The staged piece origin is padded coord (PAD - halo), so offsets
    are relative to that (rows were already stored this way; columns
    now too, enabling tight-width staging)."""
    k, d = KS[e], DS[e]
    pad = d * (k - 1) // 2
    for u in range(k):
        for v in range(k):
            slot = _SLOT_BASE[e] + u * k + v
            yield slot, d * u - pad + halo, d * v - pad + halo


def _make_schedule(idx):
    """Build the shared (all-core) round/queue schedule from gating."""
    idx = [(int(a), int(b)) for a, b in idx]
    costs = [KS[a] ** 2 + KS[b] ** 2 for a, b in idx]
    # exact-balance partition of samples onto the two PE row halves
    # (subset-sum DP over the 16 sample costs), fallback to LPT
    total = sum(costs)
    target = total // 2
    reach = {0: []}
    for s in range(B):
        upd = {}
        for v, mem in reach.items():
            nv = v + costs[s]
            if nv <= target and nv not in reach and nv not in upd:
                upd[nv] = mem + [s]
        reach.update(upd)
    bestv = max(reach)
    h0 = set(reach[bestv])
    halves = [sorted(h0), [s for s in range(B) if s not in h0]]
    loads = [bestv, total - bestv]
    # Round order per half: round 0 light (fast start), then greedily
    # prefer heavy rounds (long matmul streams cover the next round's
    # piece DMA) while spreading first-use expert-weight staging across
    # rounds (a new expert's weights compete with piece prefetch for
    # DMA bandwidth; introducing e2+e3 in one round starves the PE).
    # h1 keeps a heavy round 0: its long stream covers the early
    # prologue DMA backlog for both halves.
    for h in (0, 1):
        rest = list(halves[h])
        if h == 0:
            first = min(rest, key=lambda s: costs[s])
        else:
            first = max(rest, key=lambda s: costs[s] - 0.7 * sum(
                KS[e] ** 2 for e in idx[s]))
        rest.remove(first)
        order = [first]
        staged = set(idx[first])
        while rest:
            nxt = max(rest, key=lambda s: costs[s] - 0.7 * sum(
                KS[e] ** 2 for e in idx[s] if e not in staged))
            rest.remove(nxt)
            order.append(nxt)
            staged.update(idx[nxt])
        halves[h] = order
    rounds = [[], []]                    # per half: (sample, halo, R, roff2)
    rtot = [0, 0]                        # in parity-plane rows
    for h in (0, 1):
        for s in halves[h]:
            halo = max(HALO[e] for e in idx[s])
            r_rows = 31 + 2 * halo
            rounds[h].append((s, halo, r_rows, rtot[h]))
            rtot[h] += (r_rows + 1) // 2
    # chunk ids (output slots) + per-queue per-round chunk lists.
    # Each queue owns both chunks of each of its experts so a tap's
    # weights can serve both chunks from a single LDWEIGHTS:
    #   ch0: ea@{0,2}, eb@{1,3};  ch1: eb@{0,2}, ea@{1,3}
    # "inter" = chunk-outer interleave (round 0 / last round);
    # "pairs" = tap-outer grouping (middle rounds).
    chunk_map = []                       # cid -> (sample, expert, j, ch)
    queue_chunks = {}                    # (h, ch) -> [per-round dict]
    for h in (0, 1):
        for ch in (0, 1):
            queue_chunks[(h, ch)] = []
    for h in (0, 1):
        for (s, halo, r_rows, roff) in rounds[h]:
            ea, eb = idx[s]
            for ch in (0, 1):
                if ch == 0:
                    combos = [(ea, 0), (eb, 1), (ea, 2), (eb, 3)]
                    pair_of = [(ea, (0, 2)), (eb, (1, 3))]
                else:
                    combos = [(ea, 1), (eb, 0), (ea, 3), (eb, 2)]
                    pair_of = [(eb, (0, 2)), (ea, (1, 3))]
                cid_by_j = {}
                inter = []
                for (e, j) in combos:
                    cid = len(chunk_map)
                    chunk_map.append((s, e, j, ch))
                    cid_by_j[j] = cid
                    inter.append((e, j, cid))
                pairs = [(e, [(j, cid_by_j[j]) for j in js])
                         for (e, js) in pair_of]
                queue_chunks[(h, ch)].append(
                    dict(inter=inter, pairs=pairs))
    return dict(idx=idx, halves=halves, loads=loads, rounds=rounds,
                rtot=rtot, chunk_map=chunk_map, queue_chunks=queue_chunks)


def _dedup_ldweights(nc):
    """Remove LDWEIGHTS that reload the identical weights AP on the
    same PE tile position (the loaded weights persist in the tile's
    weight buffer, so the following matmul reuses them)."""
    removed = 0
    for f in nc.m.functions:
        for blk in f.blocks:
            last = {}
            keep = []
            changed = False
            for i in blk.instructions:
                if type(i).__name__ == "InstLdweights":
                    key = i.tile_position
                    sig = repr(i.ins[0])
                    if (last.get(key) == sig
                            and not (i.sync_info and i.sync_info.on_wait)):
                        removed += 1
                        changed = True
                        continue
                    last[key] = sig
                keep.append(i)
            if changed:
                blk.instructions = keep
    return removed


def _build_program(sched):
    import concourse.bass as bass  # noqa: F401
    import concourse.mybir as mybir
    import concourse.tile as tile
    from concourse import bacc
    from contextlib import ExitStack

    dt = mybir.dt
    nc = bacc.Bacc("TRN2", target_bir_lowering=False, debug=False,
                   num_devices=NCORES)
    xp = [nc.dram_tensor(f"xp{h}", [CIN, max(sched["rtot"][h], 1),
                                    4 * W2MAX],
                         dt.bfloat16, kind="ExternalInput") for h in (0, 1)]
    wt = nc.dram_tensor("wt", [CIN, NTAPS, CE], dt.bfloat16,
                        kind="ExternalInput")
    bnp = nc.dram_tensor("bnp", [CE, 4, 2], dt.float32, kind="ExternalInput")
    ncid = len(sched["chunk_map"])
    out = nc.dram_tensor("out", [ncid, CE, CHUNK_ROWS, WO], dt.bfloat16,
                         kind="ExternalOutput")

    with tile.TileContext(nc) as tc:
        with ExitStack() as ctx:
            consts = ctx.enter_context(tc.tile_pool(name="consts", bufs=1))
            px0 = ctx.enter_context(tc.tile_pool(name="px0", bufs=2))
            px1 = ctx.enter_context(tc.tile_pool(name="px1", bufs=2))
            piece_pools = [px0, px1]
            stage_pool = ctx.enter_context(tc.tile_pool(name="st", bufs=8))

            wtile = consts.tile([128, NTAPS, CE], dt.bfloat16)
            bntile = consts.tile([128, 4, 2], dt.float32)
            dummy = consts.tile([128, 64], dt.bfloat16)

            psum_pool = ctx.enter_context(
                tc.tile_pool(name="ps", bufs=8, space="PSUM"))

            piece_shared = [{}, {}]   # h -> round -> sbuf tile
            # DMA is bandwidth-limited during the prologue: all bulk
            # staging goes through ONE queue (gpsimd) in strict global
            # priority order — concurrent queues would steal bandwidth
            # from the latency-critical first rows. Output DMAs ride on
            # sync (as in the eviction path).
            peng = [nc.gpsimd, nc.gpsimd]     # piece staging per half
            weng = [nc.gpsimd, nc.gpsimd]     # weight staging per half
            oeng = {(0, 0): nc.sync, (0, 1): nc.sync,
                    (1, 0): nc.sync, (1, 1): nc.sync}

            def stage_piece(h, r, lo, hi, alloc=False, eng=None):
                """DMA piece rows [lo,hi) (pre-split into parity planes,
                one plane-row = 2 piece rows) of the round-r piece."""
                s, halo, r_rows, roff2 = sched["rounds"][h][r]
                p0 = h * 64
                hi = min(hi, r_rows)
                if alloc:
                    pt = piece_pools[h].tile([128, R2MAX, 4 * W2MAX],
                                             dt.bfloat16)
                    piece_shared[h][r] = pt
                else:
                    pt = piece_shared[h][r]
                if lo >= hi:
                    return
                lo2, hi2 = lo // 2, (hi + 1) // 2
                # split into <=3 descriptors so several hardware DMA
                # queues move one piece concurrently
                step = max(6, -(-(hi2 - lo2) // 3))
                for a in range(lo2, hi2, step):
                    b = min(a + step, hi2)
                    (eng or peng[h]).dma_start(
                        out=pt[p0:p0 + 64, a:b, :],
                        in_=xp[h][:, roff2 + a:roff2 + b, :])

            def stage_weights(h, e, t0=0, t1=None, eng=None):
                p0 = h * 64
                sb = _SLOT_BASE[e]
                ke = KS[e] * KS[e]
                t1 = ke if t1 is None else min(t1, ke)
                if t0 >= t1:
                    return
                (eng or weng[h]).dma_start(
                    out=wtile[p0:p0 + 64, sb + t0:sb + t1, :],
                    in_=wt[:, sb + t0:sb + t1, :])

            # ---- prologue ---------------------------------------------
            # Every engine queue issues its first DMA immediately so the
            # first transfers start in parallel (descriptor issue costs
            # ~650ns each). Priority: a dummy tile for DMA-independent
            # PE warmup, then per-half first-chunk rows + first weights.
            first_use = [[], []]       # per half: experts by first use
            for h in (0, 1):
                for (s, _h_, _r_, _o_) in sched["rounds"][h]:
                    for e in sched["idx"][s]:
                        if e not in first_use[h]:
                            first_use[h].append(e)
            halos = [sched["rounds"][h][0][1] if sched["rounds"][h] else 0
                     for h in (0, 1)]
            # warmup feed: tiny, on sync so it lands before the bulk
            # staging even starts draining
            nc.sync.dma_start(out=dummy[0:64, :], in_=wt[:, 0, :])
            # critical path, in need-order: h0's first chunks (j=0 on
            # ch0, j=1 on ch1 -> both row slabs early), then h1's
            # first slab = just the rows of the first kernel row's taps
            # (u=0 reads piece rows ro0..ro0+6), so the first matmul
            # fires as early as possible
            ro0 = [halos[h] - HALO[first_use[h][0]] if first_use[h] else 0
                   for h in (0, 1)]
            if sched["rounds"][0]:
                stage_weights(0, first_use[0][0], 0, 16)
                stage_piece(0, 0, 0, ro0[0] + 7, alloc=True)
                stage_piece(0, 0, ro0[0] + 7, 15 + 2 * halos[0])
            if sched["rounds"][1]:
                stage_weights(1, first_use[1][0], 0, 16)
                stage_piece(1, 0, 0, ro0[1] + 7, alloc=True)
            if sched["rounds"][0]:       # taps 16+ needed ~3.7us in
                stage_weights(0, first_use[0][0], 16, None)
            if sched["rounds"][1]:
                stage_piece(1, 0, ro0[1] + 7, 15 + 2 * halos[1])
            # bn params gate every eviction (needed ~6us after first MM)
            for h in (0, 1):
                nc.gpsimd.dma_start(out=bntile[h * 64:h * 64 + 64, :, :],
                                    in_=bnp.ap())
            # warm the PE clock ramp with dummy matmuls while DMA streams;
            # the warmups only need the tiny sync-queue dummy transfer,
            # so they start ~2.5us before the first staged rows land.
            # The psum slot recycles via the pool ring, it is never read.
            ps = psum_pool.tile([128, 512], dt.float32)
            for i in range(24):
                nc.tensor.matmul(ps[0:64, 0:64],
                                 dummy[0:64, :], dummy[0:64, :],
                                 start=True, stop=True,
                                 tile_position=(0, 0))
            if sched["rounds"][1]:     # h1's taps 16+
                stage_weights(1, first_use[1][0], 16, None)
            for h in (0, 1):           # second expert + rest of round 0
                if len(first_use[h]) > 1:
                    stage_weights(h, first_use[h][1])
                if sched["rounds"][h]:
                    halo0 = sched["rounds"][h][0][1]
                    stage_piece(h, 0, 15 + 2 * halo0, RMAX)
            # experts first needed in round >= 1 are staged from the pump,
            # one round ahead — keeps the prologue DMA backlog small
            first_round = [{}, {}]
            for h in (0, 1):
                for r, (s, _h_, _r_, _o_) in enumerate(sched["rounds"][h]):
                    for e in sched["idx"][s]:
                        first_round[h].setdefault(e, r)

            def queue_events(h, ch):
                p0 = h * 64            # rhs/lhsT partitions (PE rows)
                q0 = ch * 64           # psum/out partitions (PE cols)
                nrounds = len(sched["rounds"][h])
                for r, (s, halo, r_rows, roff) in enumerate(
                        sched["rounds"][h]):
                    # prefetch next round's piece + its new experts'
                    # weights, critical-first. Round 0's prefetch is
                    # DEFERRED until after the first eviction (yielded
                    # below): a fresh pool buffer has no buf-free wait,
                    # so staging issued here would flood the DMA queues
                    # and starve round 0's own critical rows.
                    pending_stage = None
                    if ch == 0 and r + 1 < nrounds:
                        h1rows = 15 + 2 * sched["rounds"][h][r + 1][1]
                        wes = [e for e, fr in first_round[h].items()
                               if fr == r + 1]
                        a_next = sched["idx"][sched["rounds"][h][r + 1][0]][0]
                        eng = nc.sync if r == 0 else None

                        def stage(h=h, r=r, h1rows=h1rows, wes=wes,
                                  a_next=a_next, eng=eng):
                            if a_next in wes:
                                stage_weights(h, a_next, 0, 16, eng=eng)
                            stage_piece(h, r + 1, 0, h1rows, alloc=True,
                                        eng=eng)
                            if a_next in wes:
                                stage_weights(h, a_next, 16, None, eng=eng)
                            for e in wes:
                                if e != a_next:
                                    stage_weights(h, e, eng=eng)
                            stage_piece(h, r + 1, h1rows, RMAX, eng=eng)
                        if r == 0:
                            pending_stage = stage
                        else:
                            yield ("dma", stage)

                    def mk_mm(j, ps, first, last, slot, ro, co,
                              h=h, r=r, p0=p0, q0=q0):
                        rl2 = (8 * j + ro) // 2
                        coff = ((ro & 1) * 2 + (co & 1)) * W2MAX + co // 2

                        def mm():
                            pt = piece_shared[h][r]
                            rhs = pt[p0:p0 + 64,
                                     rl2:rl2 + CHUNK_ROWS,
                                     coff:coff + WO]
                            lhsT = wtile[p0:p0 + 64, slot, :]
                            nc.tensor.matmul(ps[q0:q0 + 64, :], lhsT, rhs,
                                             start=first, stop=last,
                                             tile_position=(p0, q0))
                        return mm

                    def mk_evict(e, cid, ps, h=h, ch=ch, q0=q0):
                        def evict():
                            st = stage_pool.tile([128, CHUNK_ROWS, WO],
                                                 dt.bfloat16)
                            nc.scalar.activation(
                                st[q0:q0 + 64, :, :],
                                ps[q0:q0 + 64, :].rearrange(
                                    "p (a b) -> p a b", a=CHUNK_ROWS),
                                mybir.ActivationFunctionType.Gelu,
                                scale=bntile[q0:q0 + 64, e, 0:1],
                                bias=bntile[q0:q0 + 64, e, 1:2])
                            oeng[(h, ch)].dma_start(
                                out=out[cid, :, :, :],
                                in_=st[q0:q0 + 64, :, :])
                        return evict

                    rq = sched["queue_chunks"][(h, ch)][r]
                    if TAP_OUTER_MIDDLE is False or r == 0 or r == nrounds - 1:
                        # chunk-outer: each chunk's taps in sequence
                        for (e, j, cid) in rq["inter"]:
                            ps = psum_pool.tile([128, 512], dt.float32,
                                                name="ps")
                            taps = list(_tap_offsets(e, halo))
                            for t, (slot, ro, co) in enumerate(taps):
                                yield ("mm", mk_mm(j, ps, t == 0,
                                                   t == len(taps) - 1,
                                                   slot, ro, co))
                            yield ("evict", mk_evict(e, cid, ps))
                            if pending_stage is not None:
                                yield ("dma", pending_stage)
                                pending_stage = None
                    else:
                        # tap-outer: both chunks of the expert share one
                        # LDWEIGHTS per tap (the duplicate is deleted by
                        # _dedup_ldweights). Staggered by m taps so the
                        # two evictions don't bunch at the pair end:
                        # chunk A runs taps [0, m) solo, both chunks run
                        # [m, k^2) paired, A evicts (covered by B's solo
                        # finish of taps [0, m)), then B evicts (covered
                        # by the next pair's A-solo phase before its
                        # PSUM bank is rewritten).
                        for (e, jc) in rq["pairs"]:
                            pss = [psum_pool.tile([128, 512], dt.float32,
                                                  name="ps")
                                   for _ in jc]
                            taps = list(_tap_offsets(e, halo))
                            nt = len(taps)
                            m = min(5, nt // 2)
                            (ja, cida), (jb, cidb) = jc
                            for t, (slot, ro, co) in enumerate(taps):
                                yield ("mm", mk_mm(ja, pss[0], t == 0,
                                                   t == nt - 1,
                                                   slot, ro, co))
                                if t >= m:
                                    yield ("mm", mk_mm(jb, pss[1], t == m,
                                                       False,
                                                       slot, ro, co))
                            yield ("evict", mk_evict(e, cida, pss[0]))
                            for t in range(m):
                                slot, ro, co = taps[t]
                                yield ("mm", mk_mm(jb, pss[1], False,
                                                   t == m - 1,
                                                   slot, ro, co))
                            yield ("evict", mk_evict(e, cidb, pss[1]))

            queues = [queue_events(h, ch) for h in (0, 1) for ch in (0, 1)]
            live = list(queues)
            while live:
                nxt = []
                for q in live:
                    ev = next(q, None)
                    if ev is None:
                        continue
                    ev[1]()
                    nxt.append(q)
                live = nxt

    _dedup_ldweights(nc)
    nc.compile()
    return nc


def _host_gate(x, gate_w, gate_b):
    """Replicate reference gating in numpy (f64 pooling for robustness)."""
    pooled = x.astype(np.float64).mean(axis=(2, 3)).astype(np.float32)
    logits = pooled @ gate_w.T.astype(np.float32) + gate_b
    z = logits - logits.max(axis=1, keepdims=True)
    ez = np.exp(z.astype(np.float32))
    gates = ez / ez.sum(axis=1, keepdims=True)
    idx = np.argsort(-gates, axis=1, kind="stable")[:, :2]
    wsel = np.take_along_axis(gates, idx, axis=1)
    wsel = wsel / (wsel.sum(axis=1, keepdims=True) + 1e-8)
    return idx, wsel.astype(np.float32)


def _prep_inputs(x, ws, bs, bn_scale, bn_bias, bn_mean, bn_var, sched):
    bf16 = ml_dtypes.bfloat16
    xpad = np.zeros((B, CIN, HP, WP), dtype=bf16)
    xpad[:, :, PAD:PAD + H, PAD:PAD + W] = x.astype(bf16)

    # transposed weights, DMA-friendly layout [CIN, NTAPS, CE]
    wt = np.empty((CIN, NTAPS, CE), dtype=bf16)
    for e in range(4):
        k = KS[e]
        w = ws[e].astype(np.float32)  # [CE, CIN, k, k]
        wt[:, _SLOT_BASE[e]:_SLOT_BASE[e] + k * k, :] = (
            w.transpose(1, 2, 3, 0).reshape(CIN, k * k, CE).astype(bf16))

    # folded BN: z = conv*scale + shift
    inv = (bn_scale / np.sqrt(bn_var + BN_EPS)).astype(np.float32)
    shift = (np.stack(bs) * inv + bn_bias - bn_mean * inv).astype(np.float32)
    bnp = np.stack([inv, shift], axis=1)  # [4, 2, CE]
    bnp = np.ascontiguousarray(bnp.transpose(2, 0, 1))  # [CE, 4, 2]

    # per-core, per-half staged input rows (concatenated sample pieces,
    # stored at per-sample origin (PAD - halo) and split into the four
    # row/col parity planes so device-side tap matmuls read dense rows)
    xps = []
    for c in range(NCORES):
        per_half = []
        for h in (0, 1):
            buf = np.zeros((CIN, max(sched["rtot"][h], 1), 4 * W2MAX),
                           dtype=bf16)
            for (s, halo, r_rows, roff2) in sched["rounds"][h]:
                src0 = 32 * c + PAD - halo
                wpc = _piece_width(halo)
                c0 = PAD - halo
                r2 = (r_rows + 1) // 2
                tmp = np.zeros((CIN, 2 * r2, 2 * W2MAX), dtype=bf16)
                tmp[:, :r_rows, :wpc] = (
                    xpad[s, :, src0:src0 + r_rows, c0:c0 + wpc])
                # (cin, 2*r2, 2*W2) -> (cin, r2, pr, pc, c2) flattened
                planes = tmp.reshape(CIN, r2, 2, W2MAX, 2).transpose(
                    0, 1, 2, 4, 3).reshape(CIN, r2, 4 * W2MAX)
                buf[:, roff2:roff2 + r2, :] = planes
            per_half.append(buf)
        xps.append(per_half)
    return xps, wt, bnp


def _get_program(idx):
    key = np.asarray(idx, np.int64).tobytes()
    if key not in _CACHE:
        sched = _make_schedule(idx)
        _CACHE[key] = (sched, _build_program(sched))
    return _CACHE[key]


def run(inputs, trace=False):
    from concourse import bass_utils

    x = np.asarray(inputs["x"], dtype=np.float32)
    ws = [np.asarray(inputs[f"w{i}"], dtype=np.float32) for i in range(4)]
    bs = [np.asarray(inputs[f"b{i}"], dtype=np.float32) for i in range(4)]
    bn_scale = np.asarray(inputs["bn_scale"], dtype=np.float32)
    bn_bias = np.asarray(inputs["bn_bias"], dtype=np.float32)
    bn_mean = np.asarray(inputs["bn_mean"], dtype=np.float32)
    bn_var = np.asarray(inputs["bn_var"], dtype=np.float32)
    gate_w = np.asarray(inputs["gate_w"], dtype=np.float32)
    gate_b = np.asarray(inputs["gate_b"], dtype=np.float32)

    idx, wsel = _host_gate(x, gate_w, gate_b)
    sched, nc = _get_program(idx)
    xps, wt, bnp = _prep_inputs(x, ws, bs, bn_scale, bn_bias, bn_mean,
                                bn_var, sched)
    in_maps = []
    for c in range(NCORES):
        in_maps.append({
            "xp0": xps[c][0],
            "xp1": xps[c][1],
            "wt": wt,
            "bnp": bnp,
        })
    res = bass_utils.run_bass_kernel_spmd(
        nc, in_maps, core_ids=list(range(NCORES)), trace=trace)

    # assemble: scatter band chunks, apply top-2 gate weights, concat
    outf = np.empty((B, 2 * CE, HO, WO), dtype=np.float32)
    for c in range(NCORES):
        o = np.asarray(res.results[c]["out"], dtype=np.float32)
        for cid, (s, e, j, _ch) in enumerate(sched["chunk_map"]):
            sl = 0 if sched["idx"][s][0] == e else 1
            r0 = BAND * c + CHUNK_ROWS * j
            outf[s, sl * CE:(sl + 1) * CE, r0:r0 + CHUNK_ROWS, :] = (
                o[cid] * wsel[s, sl])
    return outf, res


def kernel(**inputs):
    outf, _ = run(inputs, trace=False)
    return outf


# revision 32
# speedup vs baseline: 1.0528x; 1.0528x over previous
"""MoE downsample kernel for 8 TRN2 NeuronCores — top-2 sparse.

The reference computes all 4 experts densely, but only the top-2 gated
experts per sample contribute to the output. Gating depends only on the
input mean-pool, so it is computed on host BEFORE compiling the device
program; the program then contains matmuls only for the selected
(sample, expert) pairs (~half the dense FLOPs for typical gatings).

Sharding: every selected (sample, expert) conv is band-sharded across
all 8 cores — core c computes output rows [16c, 16c+16). All cores
therefore execute an IDENTICAL instruction stream (SPMD-safe); only the
staged input rows differ per core. Within a core, samples are processed
in rounds streamed through SBUF: samples are exactly balanced
(subset-sum over gating costs) onto the two PE row-halves (partitions
0-63 / 64-127), and each sample's two experts are split across the two
PE col-halves so four 64x64 tile_position matmuls run concurrently
(full 128x128 array). Each strided dilated
conv is decomposed into k*k "tap" matmuls accumulated in PSUM over
512-pixel chunks; BN + conv-bias + GELU fuse into the ScalarE PSUM
eviction. Top-2 weighting and concat run on host.

The PE weight-load path (one LDWEIGHTS per matmul, ~53ns per 64-col
load) is co-saturated with the moving-data path when every 512-px
matmul reloads its tap weights. Middle rounds therefore run TAP-OUTER:
each queue processes its two chunks of an (s, e) pair per tap
back-to-back, and a post-schedule pass deletes the second (redundant)
LDWEIGHTS so each tap's weights are loaded once per queue instead of
twice. Round 0 stays chunk-outer so the first matmul only needs a few
staged rows; the last round stays chunk-outer to stagger the final
GELU evictions into the matmul stream (shorter tail).
"""

import numpy as np
import ml_dtypes

KS = [3, 5, 7, 9]
DS = [1, 2, 3, 4]
HALO = [d * (k - 1) // 2 for k, d in zip(KS, DS)]  # [1, 4, 9, 16]
BN_EPS = 1e-5
B, CIN, H, W = 16, 64, 256, 256
CE = 64
PAD = 16           # left/top pad (max halo); right/bottom needs 15
HP = WP = PAD + 256 + 15   # 287
HO = WO = 128
NCORES = 8
BAND = 16          # output rows per core per (sample, expert)
CHUNK_ROWS = 4     # output rows per 512-px PSUM chunk
NCHUNK = BAND // CHUNK_ROWS   # 4 chunks per (sample, expert) band
RMAX = 31 + 2 * max(HALO)     # 63 input rows per staged piece (max)
R2MAX = (RMAX + 1) // 2       # 32 rows per parity plane
W2MAX = 144                   # cols per parity plane (ceil(287/2) rounded)
NTAPS = sum(k * k for k in KS)  # 164
# chunk-outer everywhere: tap-outer + LDWEIGHTS dedup measured neutral
# on the PE (the weight path is not the binding resource), and its
# eviction bunching cost more than the reuse saved
TAP_OUTER_MIDDLE = False

# tap slot base per expert in the packed weight tensor
_SLOT_BASE = np.cumsum([0] + [k * k for k in KS]).tolist()

_CACHE = {}


def _piece_width(halo):
    """Staged piece columns: window cols span [0, 2*halo + 254]."""
    return 2 * halo + 255


def _plane_width(halo):
    """Per-parity-plane columns: ceil(piece_width / 2)."""
    return halo + 128


def _tap_offsets(e, halo):
    """Yield (slot, row_off, col_off) in piece coords for expert e.

    The staged piece origin is padded coord (PAD - halo), so offsets
    are relative to that (rows were already stored this way; columns
    now too, enabling tight-width staging)."""
    k, d = KS[e], DS[e]
    pad = d * (k - 1) // 2
    for u in range(k):
        for v in range(k):
            slot = _SLOT_BASE[e] + u * k + v
            yield slot, d * u - pad + halo, d * v - pad + halo


def _make_schedule(idx):
    """Build the shared (all-core) round/queue schedule from gating."""
    idx = [(int(a), int(b)) for a, b in idx]
    costs = [KS[a] ** 2 + KS[b] ** 2 for a, b in idx]
    # exact-balance partition of samples onto the two PE row halves
    # (subset-sum DP over the 16 sample costs), fallback to LPT
    total = sum(costs)
    target = total // 2
    reach = {0: []}
    for s in range(B):
        upd = {}
        for v, mem in reach.items():
            nv = v + costs[s]
            if nv <= target and nv not in reach and nv not in upd:
                upd[nv] = mem + [s]
        reach.update(upd)
    bestv = max(reach)
    h0 = set(reach[bestv])
    halves = [sorted(h0), [s for s in range(B) if s not in h0]]
    loads = [bestv, total - bestv]
    # Round order per half: round 0 light (fast start), then greedily
    # prefer heavy rounds (long matmul streams cover the next round's
    # piece DMA) while spreading first-use expert-weight staging across
    # rounds (a new expert's weights compete with piece prefetch for
    # DMA bandwidth; introducing e2+e3 in one round starves the PE).
    # h1 keeps a heavy round 0: its long stream covers the early
    # prologue DMA backlog for both halves.
    for h in (0, 1):
        rest = list(halves[h])
        if h == 0:
            first = min(rest, key=lambda s: costs[s])
        else:
            first = max(rest, key=lambda s: costs[s] - 0.7 * sum(
                KS[e] ** 2 for e in idx[s]))
        rest.remove(first)
        order = [first]
        staged = set(idx[first])
        while rest:
            nxt = max(rest, key=lambda s: costs[s] - 0.7 * sum(
                KS[e] ** 2 for e in idx[s] if e not in staged))
            rest.remove(nxt)
            order.append(nxt)
            staged.update(idx[nxt])
        halves[h] = order
    rounds = [[], []]                    # per half: (sample, halo, R, roff2)
    rtot = [0, 0]                        # in parity-plane rows
    for h in (0, 1):
        for s in halves[h]:
            halo = max(HALO[e] for e in idx[s])
            r_rows = 31 + 2 * halo
            rounds[h].append((s, halo, r_rows, rtot[h]))
            rtot[h] += (r_rows + 1) // 2
    # chunk ids (output slots) + per-queue per-round chunk lists.
    # Each queue owns both chunks of each of its experts so a tap's
    # weights can serve both chunks from a single LDWEIGHTS:
    #   ch0: ea@{0,2}, eb@{1,3};  ch1: eb@{0,2}, ea@{1,3}
    # "inter" = chunk-outer interleave (round 0 / last round);
    # "pairs" = tap-outer grouping (middle rounds).
    chunk_map = []                       # cid -> (sample, expert, j, ch)
    queue_chunks = {}                    # (h, ch) -> [per-round dict]
    for h in (0, 1):
        for ch in (0, 1):
            queue_chunks[(h, ch)] = []
    for h in (0, 1):
        for (s, halo, r_rows, roff) in rounds[h]:
            ea, eb = idx[s]
            for ch in (0, 1):
                if ch == 0:
                    combos = [(ea, 0), (eb, 1), (ea, 2), (eb, 3)]
                    pair_of = [(ea, (0, 2)), (eb, (1, 3))]
                else:
                    combos = [(ea, 1), (eb, 0), (ea, 3), (eb, 2)]
                    pair_of = [(eb, (0, 2)), (ea, (1, 3))]
                cid_by_j = {}
                inter = []
                for (e, j) in combos:
                    cid = len(chunk_map)
                    chunk_map.append((s, e, j, ch))
                    cid_by_j[j] = cid
                    inter.append((e, j, cid))
                pairs = [(e, [(j, cid_by_j[j]) for j in js])
                         for (e, js) in pair_of]
                queue_chunks[(h, ch)].append(
                    dict(inter=inter, pairs=pairs))
    return dict(idx=idx, halves=halves, loads=loads, rounds=rounds,
                rtot=rtot, chunk_map=chunk_map, queue_chunks=queue_chunks)


def _dedup_ldweights(nc):
    """Remove LDWEIGHTS that reload the identical weights AP on the
    same PE tile position (the loaded weights persist in the tile's
    weight buffer, so the following matmul reuses them)."""
    removed = 0
    for f in nc.m.functions:
        for blk in f.blocks:
            last = {}
            keep = []
            changed = False
            for i in blk.instructions:
                if type(i).__name__ == "InstLdweights":
                    key = i.tile_position
                    sig = repr(i.ins[0])
                    if (last.get(key) == sig
                            and not (i.sync_info and i.sync_info.on_wait)):
                        removed += 1
                        changed = True
                        continue
                    last[key] = sig
                keep.append(i)
            if changed:
                blk.instructions = keep
    return removed


def _build_program(sched):
    import concourse.bass as bass  # noqa: F401
    import concourse.mybir as mybir
    import concourse.tile as tile
    from concourse import bacc
    from contextlib import ExitStack

    dt = mybir.dt
    nc = bacc.Bacc("TRN2", target_bir_lowering=False, debug=False,
                   num_devices=NCORES)
    xp = [nc.dram_tensor(f"xp{h}", [CIN, max(sched["rtot"][h], 1),
                                    4 * W2MAX],
                         dt.bfloat16, kind="ExternalInput") for h in (0, 1)]
    wt = nc.dram_tensor("wt", [CIN, NTAPS, CE], dt.bfloat16,
                        kind="ExternalInput")
    bnp = nc.dram_tensor("bnp", [CE, 4, 2], dt.float32, kind="ExternalInput")
    ncid = len(sched["chunk_map"])
    out = nc.dram_tensor("out", [ncid, CE, CHUNK_ROWS, WO], dt.bfloat16,
                         kind="ExternalOutput")

    with tile.TileContext(nc) as tc:
        with ExitStack() as ctx:
            consts = ctx.enter_context(tc.tile_pool(name="consts", bufs=1))
            px0 = ctx.enter_context(tc.tile_pool(name="px0", bufs=2))
            px1 = ctx.enter_context(tc.tile_pool(name="px1", bufs=2))
            piece_pools = [px0, px1]
            stage_pool = ctx.enter_context(tc.tile_pool(name="st", bufs=8))

            wtile = consts.tile([128, NTAPS, CE], dt.bfloat16)
            bntile = consts.tile([128, 4, 2], dt.float32)
            dummy = consts.tile([128, 64], dt.bfloat16)

            psum_pool = ctx.enter_context(
                tc.tile_pool(name="ps", bufs=8, space="PSUM"))

            piece_shared = [{}, {}]   # h -> round -> sbuf tile
            # DMA is bandwidth-limited during the prologue: all bulk
            # staging goes through ONE queue (gpsimd) in strict global
            # priority order — concurrent queues would steal bandwidth
            # from the latency-critical first rows. Output DMAs ride on
            # sync (as in the eviction path).
            peng = [nc.gpsimd, nc.gpsimd]     # piece staging per half
            weng = [nc.gpsimd, nc.gpsimd]     # weight staging per half
            oeng = {(0, 0): nc.sync, (0, 1): nc.sync,
                    (1, 0): nc.sync, (1, 1): nc.sync}

            def stage_piece(h, r, lo, hi, alloc=False, eng=None):
                """DMA piece rows [lo,hi) (pre-split into parity planes,
                one plane-row = 2 piece rows) of the round-r piece."""
                s, halo, r_rows, roff2 = sched["rounds"][h][r]
                p0 = h * 64
                hi = min(hi, r_rows)
                if alloc:
                    pt = piece_pools[h].tile([128, R2MAX, 4 * W2MAX],
                                             dt.bfloat16)
                    piece_shared[h][r] = pt
                else:
                    pt = piece_shared[h][r]
                if lo >= hi:
                    return
                lo2, hi2 = lo // 2, (hi + 1) // 2
                # split into <=3 descriptors so several hardware DMA
                # queues move one piece concurrently
                step = max(6, -(-(hi2 - lo2) // 3))
                for a in range(lo2, hi2, step):
                    b = min(a + step, hi2)
                    (eng or peng[h]).dma_start(
                        out=pt[p0:p0 + 64, a:b, :],
                        in_=xp[h][:, roff2 + a:roff2 + b, :])

            def stage_weights(h, e, t0=0, t1=None, eng=None):
                p0 = h * 64
                sb = _SLOT_BASE[e]
                ke = KS[e] * KS[e]
                t1 = ke if t1 is None else min(t1, ke)
                if t0 >= t1:
                    return
                (eng or weng[h]).dma_start(
                    out=wtile[p0:p0 + 64, sb + t0:sb + t1, :],
                    in_=wt[:, sb + t0:sb + t1, :])

            # ---- prologue ---------------------------------------------
            # Every engine queue issues its first DMA immediately so the
            # first transfers start in parallel (descriptor issue costs
            # ~650ns each). Priority: a dummy tile for DMA-independent
            # PE warmup, then per-half first-chunk rows + first weights.
            first_use = [[], []]       # per half: experts by first use
            for h in (0, 1):
                for (s, _h_, _r_, _o_) in sched["rounds"][h]:
                    for e in sched["idx"][s]:
                        if e not in first_use[h]:
                            first_use[h].append(e)
            halos = [sched["rounds"][h][0][1] if sched["rounds"][h] else 0
                     for h in (0, 1)]
            # warmup feed: tiny, on sync so it lands before the bulk
            # staging even starts draining
            nc.sync.dma_start(out=dummy[0:64, :], in_=wt[:, 0, :])
            # critical path, in need-order: h0's first chunks (j=0 on
            # ch0, j=1 on ch1 -> both row slabs early), then h1's
            # first slab = just the rows of the first kernel row's taps
            # (u=0 reads piece rows ro0..ro0+6), so the first matmul
            # fires as early as possible
            ro0 = [halos[h] - HALO[first_use[h][0]] if first_use[h] else 0
                   for h in (0, 1)]
            # tiny first slabs ride on sync (parallel to gpsimd's weight
            # transfers) so the very first matmuls unblock early; bulk
            # staging stays strictly ordered on gpsimd
            if sched["rounds"][0]:
                stage_weights(0, first_use[0][0], 0, 16)
                stage_piece(0, 0, 0, ro0[0] + 7, alloc=True, eng=nc.sync)
                stage_piece(0, 0, ro0[0] + 7, 15 + 2 * halos[0])
            if sched["rounds"][1]:
                stage_weights(1, first_use[1][0], 0, 16)
                stage_piece(1, 0, 0, ro0[1] + 7, alloc=True, eng=nc.sync)
            if sched["rounds"][0]:       # taps 16+ needed ~3.7us in
                stage_weights(0, first_use[0][0], 16, None)
            if sched["rounds"][1]:
                stage_piece(1, 0, ro0[1] + 7, 15 + 2 * halos[1])
            # bn params gate every eviction (needed ~6us after first MM)
            for h in (0, 1):
                nc.gpsimd.dma_start(out=bntile[h * 64:h * 64 + 64, :, :],
                                    in_=bnp.ap())
            # warm the PE clock ramp with dummy matmuls while DMA streams;
            # the warmups only need the tiny sync-queue dummy transfer,
            # so they start ~2.5us before the first staged rows land.
            # The psum slot recycles via the pool ring, it is never read.
            ps = psum_pool.tile([128, 512], dt.float32)
            for i in range(24):
                nc.tensor.matmul(ps[0:64, 0:64],
                                 dummy[0:64, :], dummy[0:64, :],
                                 start=True, stop=True,
                                 tile_position=(0, 0))
            if sched["rounds"][1]:     # h1's taps 16+
                stage_weights(1, first_use[1][0], 16, None)
            for h in (0, 1):           # second expert + rest of round 0
                if len(first_use[h]) > 1:
                    stage_weights(h, first_use[h][1])
                if sched["rounds"][h]:
                    halo0 = sched["rounds"][h][0][1]
                    stage_piece(h, 0, 15 + 2 * halo0, RMAX)
            # experts first needed in round >= 1 are staged from the pump,
            # one round ahead — keeps the prologue DMA backlog small
            first_round = [{}, {}]
            for h in (0, 1):
                for r, (s, _h_, _r_, _o_) in enumerate(sched["rounds"][h]):
                    for e in sched["idx"][s]:
                        first_round[h].setdefault(e, r)

            def queue_events(h, ch):
                p0 = h * 64            # rhs/lhsT partitions (PE rows)
                q0 = ch * 64           # psum/out partitions (PE cols)
                nrounds = len(sched["rounds"][h])
                for r, (s, halo, r_rows, roff) in enumerate(
                        sched["rounds"][h]):
                    # prefetch next round's piece + its new experts'
                    # weights, critical-first. Round 0's prefetch is
                    # DEFERRED until after the first eviction (yielded
                    # below): a fresh pool buffer has no buf-free wait,
                    # so staging issued here would flood the DMA queues
                    # and starve round 0's own critical rows.
                    pending_stage = None
                    if ch == 0 and r + 1 < nrounds:
                        halo1 = sched["rounds"][h][r + 1][1]
                        h1rows = 15 + 2 * halo1
                        wes = [e for e, fr in first_round[h].items()
                               if fr == r + 1]
                        a_next = sched["idx"][sched["rounds"][h][r + 1][0]][0]
                        # minimal head: rows for both queues' first
                        # chunks (j=0 and j=1 of a_next)
                        hd = halo1 - HALO[a_next] + 15

                        def stage(h=h, r=r, hd=hd, h1rows=h1rows, wes=wes,
                                  a_next=a_next):
                            if a_next in wes:
                                stage_weights(h, a_next, 0, 16)
                            stage_piece(h, r + 1, 0, hd, alloc=True)
                            if a_next in wes:
                                stage_weights(h, a_next, 16, None)
                            stage_piece(h, r + 1, hd, h1rows)
                            for e in wes:
                                if e != a_next:
                                    stage_weights(h, e)
                            stage_piece(h, r + 1, h1rows, RMAX)
                        yield ("dma", stage)

                    def mk_mm(j, ps, first, last, slot, ro, co,
                              h=h, r=r, p0=p0, q0=q0):
                        rl2 = (8 * j + ro) // 2
                        coff = ((ro & 1) * 2 + (co & 1)) * W2MAX + co // 2

                        def mm():
                            pt = piece_shared[h][r]
                            rhs = pt[p0:p0 + 64,
                                     rl2:rl2 + CHUNK_ROWS,
                                     coff:coff + WO]
                            lhsT = wtile[p0:p0 + 64, slot, :]
                            nc.tensor.matmul(ps[q0:q0 + 64, :], lhsT, rhs,
                                             start=first, stop=last,
                                             tile_position=(p0, q0))
                        return mm

                    def mk_evict(e, cid, ps, h=h, ch=ch, q0=q0):
                        def evict():
                            st = stage_pool.tile([128, CHUNK_ROWS, WO],
                                                 dt.bfloat16)
                            nc.scalar.activation(
                                st[q0:q0 + 64, :, :],
                                ps[q0:q0 + 64, :].rearrange(
                                    "p (a b) -> p a b", a=CHUNK_ROWS),
                                mybir.ActivationFunctionType.Gelu,
                                scale=bntile[q0:q0 + 64, e, 0:1],
                                bias=bntile[q0:q0 + 64, e, 1:2])
                            oeng[(h, ch)].dma_start(
                                out=out[cid, :, :, :],
                                in_=st[q0:q0 + 64, :, :])
                        return evict

                    rq = sched["queue_chunks"][(h, ch)][r]
                    if TAP_OUTER_MIDDLE is False or r == 0 or r == nrounds - 1:
                        # chunk-outer: each chunk's taps in sequence
                        for (e, j, cid) in rq["inter"]:
                            ps = psum_pool.tile([128, 512], dt.float32,
                                                name="ps")
                            taps = list(_tap_offsets(e, halo))
                            for t, (slot, ro, co) in enumerate(taps):
                                yield ("mm", mk_mm(j, ps, t == 0,
                                                   t == len(taps) - 1,
                                                   slot, ro, co))
                            yield ("evict", mk_evict(e, cid, ps))
                    else:
                        # tap-outer: both chunks of the expert share one
                        # LDWEIGHTS per tap (the duplicate is deleted by
                        # _dedup_ldweights). Staggered by m taps so the
                        # two evictions don't bunch at the pair end:
                        # chunk A runs taps [0, m) solo, both chunks run
                        # [m, k^2) paired, A evicts (covered by B's solo
                        # finish of taps [0, m)), then B evicts (covered
                        # by the next pair's A-solo phase before its
                        # PSUM bank is rewritten).
                        for (e, jc) in rq["pairs"]:
                            pss = [psum_pool.tile([128, 512], dt.float32,
                                                  name="ps")
                                   for _ in jc]
                            taps = list(_tap_offsets(e, halo))
                            nt = len(taps)
                            m = min(5, nt // 2)
                            (ja, cida), (jb, cidb) = jc
                            for t, (slot, ro, co) in enumerate(taps):
                                yield ("mm", mk_mm(ja, pss[0], t == 0,
                                                   t == nt - 1,
                                                   slot, ro, co))
                                if t >= m:
                                    yield ("mm", mk_mm(jb, pss[1], t == m,
                                                       False,
                                                       slot, ro, co))
                            yield ("evict", mk_evict(e, cida, pss[0]))
                            for t in range(m):
                                slot, ro, co = taps[t]
                                yield ("mm", mk_mm(jb, pss[1], False,
                                                   t == m - 1,
                                                   slot, ro, co))
                            yield ("evict", mk_evict(e, cidb, pss[1]))

            queues = [queue_events(h, ch) for h in (0, 1) for ch in (0, 1)]
            live = list(queues)
            while live:
                nxt = []
                for q in live:
                    ev = next(q, None)
                    if ev is None:
                        continue
                    ev[1]()
                    nxt.append(q)
                live = nxt

    _dedup_ldweights(nc)
    nc.compile()
    return nc


def _host_gate(x, gate_w, gate_b):
    """Replicate reference gating in numpy (f64 pooling for robustness)."""
    pooled = x.astype(np.float64).mean(axis=(2, 3)).astype(np.float32)
    logits = pooled @ gate_w.T.astype(np.float32) + gate_b
    z = logits - logits.max(axis=1, keepdims=True)
    ez = np.exp(z.astype(np.float32))
    gates = ez / ez.sum(axis=1, keepdims=True)
    idx = np.argsort(-gates, axis=1, kind="stable")[:, :2]
    wsel = np.take_along_axis(gates, idx, axis=1)
    wsel = wsel / (wsel.sum(axis=1, keepdims=True) + 1e-8)
    return idx, wsel.astype(np.float32)


def _prep_inputs(x, ws, bs, bn_scale, bn_bias, bn_mean, bn_var, sched):
    bf16 = ml_dtypes.bfloat16
    xpad = np.zeros((B, CIN, HP, WP), dtype=bf16)
    xpad[:, :, PAD:PAD + H, PAD:PAD + W] = x.astype(bf16)

    # transposed weights, DMA-friendly layout [CIN, NTAPS, CE]
    wt = np.empty((CIN, NTAPS, CE), dtype=bf16)
    for e in range(4):
        k = KS[e]
        w = ws[e].astype(np.float32)  # [CE, CIN, k, k]
        wt[:, _SLOT_BASE[e]:_SLOT_BASE[e] + k * k, :] = (
            w.transpose(1, 2, 3, 0).reshape(CIN, k * k, CE).astype(bf16))

    # folded BN: z = conv*scale + shift
    inv = (bn_scale / np.sqrt(bn_var + BN_EPS)).astype(np.float32)
    shift = (np.stack(bs) * inv + bn_bias - bn_mean * inv).astype(np.float32)
    bnp = np.stack([inv, shift], axis=1)  # [4, 2, CE]
    bnp = np.ascontiguousarray(bnp.transpose(2, 0, 1))  # [CE, 4, 2]

    # per-core, per-half staged input rows (concatenated sample pieces,
    # stored at per-sample origin (PAD - halo) and split into the four
    # row/col parity planes so device-side tap matmuls read dense rows)
    xps = []
    for c in range(NCORES):
        per_half = []
        for h in (0, 1):
            buf = np.zeros((CIN, max(sched["rtot"][h], 1), 4 * W2MAX),
                           dtype=bf16)
            for (s, halo, r_rows, roff2) in sched["rounds"][h]:
                src0 = 32 * c + PAD - halo
                wpc = _piece_width(halo)
                c0 = PAD - halo
                r2 = (r_rows + 1) // 2
                tmp = np.zeros((CIN, 2 * r2, 2 * W2MAX), dtype=bf16)
                tmp[:, :r_rows, :wpc] = (
                    xpad[s, :, src0:src0 + r_rows, c0:c0 + wpc])
                # (cin, 2*r2, 2*W2) -> (cin, r2, pr, pc, c2) flattened
                planes = tmp.reshape(CIN, r2, 2, W2MAX, 2).transpose(
                    0, 1, 2, 4, 3).reshape(CIN, r2, 4 * W2MAX)
                buf[:, roff2:roff2 + r2, :] = planes
            per_half.append(buf)
        xps.append(per_half)
    return xps, wt, bnp


def _get_program(idx):
    key = np.asarray(idx, np.int64).tobytes()
    if key not in _CACHE:
        sched = _make_schedule(idx)
        _CACHE[key] = (sched, _build_program(sched))
    return _CACHE[key]


def run(inputs, trace=False):
    from concourse import bass_utils

    x = np.asarray(inputs["x"], dtype=np.float32)
    ws = [np.asarray(inputs[f"w{i}"], dtype=np.float32) for i in range(4)]
    bs = [np.asarray(inputs[f"b{i}"], dtype=np.float32) for i in range(4)]
    bn_scale = np.asarray(inputs["bn_scale"], dtype=np.float32)
    bn_bias = np.asarray(inputs["bn_bias"], dtype=np.float32)
    bn_mean = np.asarray(inputs["bn_mean"], dtype=np.float32)
    bn_var = np.asarray(inputs["bn_var"], dtype=np.float32)
    gate_w = np.asarray(inputs["gate_w"], dtype=np.float32)
    gate_b = np.asarray(inputs["gate_b"], dtype=np.float32)

    idx, wsel = _host_gate(x, gate_w, gate_b)
    sched, nc = _get_program(idx)
    xps, wt, bnp = _prep_inputs(x, ws, bs, bn_scale, bn_bias, bn_mean,
                                bn_var, sched)
    in_maps = []
    for c in range(NCORES):
        in_maps.append({
            "xp0": xps[c][0],
            "xp1": xps[c][1],
            "wt": wt,
            "bnp": bnp,
        })
    res = bass_utils.run_bass_kernel_spmd(
        nc, in_maps, core_ids=list(range(NCORES)), trace=trace)

    # assemble: scatter band chunks, apply top-2 gate weights, concat
    outf = np.empty((B, 2 * CE, HO, WO), dtype=np.float32)
    for c in range(NCORES):
        o = np.asarray(res.results[c]["out"], dtype=np.float32)
        for cid, (s, e, j, _ch) in enumerate(sched["chunk_map"]):
            sl = 0 if sched["idx"][s][0] == e else 1
            r0 = BAND * c + CHUNK_ROWS * j
            outf[s, sl * CE:(sl + 1) * CE, r0:r0 + CHUNK_ROWS, :] = (
                o[cid] * wsel[s, sl])
    return outf, res


def kernel(**inputs):
    outf, _ = run(inputs, trace=False)
    return outf
